# revision 1
# baseline (speedup 1.0000x reference)
"""CapsuleLayer dynamic-routing kernel for 8 Trainium2 NeuronCores.

Math (per sample):
    u_hat[n,m,c] = sum_d x[n,d] W[d,m,c]           (never materialized)
    routing r=1..3:
        c = softmax_n(b)            -> c_unnorm = exp(b), Z[m] = sum_n c_unnorm
        s[m,c] = sum_n c[n,m] u_hat[n,m,c] = (sum_d T[m,d] W[d,m,c]) / Z[m]
                 where T[m,d] = sum_n c_unnorm[n,m] x[n,d]
        v = squash(s)
        b += sum_c v[m,c] u_hat[n,m,c] = x @ P_r.T  where P_r[m,d] = sum_c v W
    With Q_r = sum_{r'<=r} P_r', the logits are always b_r = x @ Q_r.T, so we
    accumulate Q (tiny) instead of b (big).  exp() overflow-safe without the
    max-subtraction: |b| <~ 60 << 88.

Sharding: batch 64 -> 8 samples/core, fully independent.
"""

import os
import sys

import numpy as np

for _p in ("/opt/trn_rl_repo", os.path.expanduser("~/.axon_site/_ro/trn_rl_repo")):
    if os.path.isdir(_p) and _p not in sys.path:
        sys.path.insert(0, _p)

import concourse.bass as bass
import concourse.tile as tile
from concourse import mybir
from concourse.vector_clock import ScopedClock, VectorClock
from bass_rust import N_PROCS


class _SplitDrainTC(tile.TileContext):
    """TileContext whose exit drain is split into several drains with few
    sem waits each: walrus rejects a single drain waiting on >~8 sems."""

    def _drain_and_barrier(self, tick_clock, wait_clock):
        gc = tick_clock.global_clock
        CH = 1
        for i in range(0, N_PROCS, CH):
            sub = VectorClock(
                [gc[p] if i <= p < i + CH else 0 for p in range(N_PROCS)]
            )
            drain_inst = self.nc.sync.drain()
            wait_clock.add_sem_waits(
                drain_inst.ins, ScopedClock({None: sub})
            )
        self.nc.all_engine_barrier()
        assert self.sems is not None
        popped = self.nc._tile_sem_poison_stack.pop()
        assert popped is self._sem_poison
        self.nc.clear_and_free_semaphores(list(self.sems.allocated().values()))
        self.nc.all_engine_barrier()

B, N, D, M, C = 64, 2048, 16, 32, 16
NCORES = 8
BL = B // NCORES          # samples per core = 8
G = BL // 4               # sample groups of 4 -> 2
NCHUNK = N // 128         # 16
NWIN = N // 256           # 8 transpose windows of 256
ROUTINGS = 3
EPS = 1e-7
F32 = mybir.dt.float32
BF16 = mybir.dt.bfloat16
ALU = mybir.AluOpType
ACTF = mybir.ActivationFunctionType
RSQRT_MAGIC = 0x5F3759DF


def _bcast(ap, idx, num):
    """Insert a stride-0 free dim of size `num` at free-dim position idx."""
    dims = list(ap.ap)
    dims.insert(1 + idx, [0, num])
    return bass.AP(ap.tensor, ap.offset, dims)


def build_bass():
    nc = bass.Bass()
    x_in = nc.declare_dram_parameter("x", [BL, N, D], F32, isOutput=False)
    w_in = nc.declare_dram_parameter("w", [D, M, C], F32, isOutput=False)
    v_out = nc.declare_dram_parameter("v", [BL, M, C], F32, isOutput=True)

    with _SplitDrainTC(nc) as tc:
        _emit(tc, x_in, w_in, v_out)
    return nc


def _emit(tc, x_in, w_in, v_out):
    nc = tc.nc
    P = 128

    from contextlib import ExitStack

    ctx = ExitStack()
    const = ctx.enter_context(tc.tile_pool(name="const", bufs=1))
    ld = ctx.enter_context(tc.tile_pool(name="ld", bufs=2))
    work = ctx.enter_context(tc.tile_pool(name="work", bufs=2))
    small = ctx.enter_context(tc.tile_pool(name="small", bufs=4))
    psum_b = ctx.enter_context(tc.tile_pool(name="psum_b", bufs=2, space="PSUM"))
    psum_t = ctx.enter_context(tc.tile_pool(name="psum_t", bufs=3, space="PSUM"))

    x = x_in[:]
    w = w_in[:]
    vout = v_out[:]

    # ---------------- constants / input staging ----------------
    # xq[g][p, k, si, 0:16] = x[4g+si, 128k+p, d]; [..,16] = 1.0 (Z column).
    # Staged via a raw tile + one flat 2D copy so every consumer instruction
    # waits on a single DVE semaphore and lowers to a 2D (wait-slot-rich)
    # encoding: walrus rejects >=3D instructions with multiple sync waits.
    xq = []
    for g in range(G):
        xq_raw = const.tile(
            [P, NCHUNK, 4, D], F32, name=f"xq_raw_{g}", tag=f"xq_raw_{g}"
        )
        eng = nc.scalar if g == 0 else nc.sync
        for si in range(4):
            eng.dma_start(
                out=xq_raw[:, :, si, :],
                in_=x[4 * g + si].rearrange("(k p) d -> p k d", p=P),
            )
        xqg = const.tile([P, NCHUNK, 4, D + 1], F32, name=f"xq_{g}", tag=f"xq_{g}")
        nc.gpsimd.memset(xqg[:, :, :, D : D + 1], 1.0)
        for si in range(4):
            nc.gpsimd.tensor_copy(
                out=xqg[:, :, si, 0:D], in_=xq_raw[:, :, si, :]
            )
        xq.append(xqg)

    # W4[32g+m, d, c] = W[d, m, c] replicated over the 4 samples of a group
    w1 = const.tile([32, D, C], F32)
    nc.scalar.dma_start(out=w1, in_=w.rearrange("d m c -> m d c"))
    w4 = const.tile([P, D, C], F32)
    for gi in range(4):
        nc.vector.tensor_copy(out=w4[32 * gi : 32 * gi + 32, :, :], in_=w1)

    cconst = const.tile([P, P], F32)
    nc.gpsimd.memset(cconst, 1.0 / N)
    zz = const.tile([P, 2], BF16)
    nc.gpsimd.memset(zz, 0.0)
    zb = const.tile([P, P], BF16)
    nc.gpsimd.memset(zb, 0.0)
    wtz = psum_t.tile([P, 72], F32, name="wtz", tag="tz")
    nc.tensor.matmul(out=wtz[:, 68:70], lhsT=zb[:, :], rhs=zz[:, :],
                     start=True, stop=True)

    # xT4[g][32s'+d, n] = x[4g+s', n, d]  (rows with d in [16,32) are zero)
    # built via DVE 32x32 block transpose of [32s'+nl, (win, wv, 32dpad)] tiles
    xt4 = [const.tile([P, N], F32, name=f"xt4_{g}", tag=f"xt4_{g}") for g in range(G)]
    for g in range(G):
        lt = ld.tile([P, NWIN, 8, 32], F32, name="ldt", tag="ldt")
        for si in range(4):
            eng = nc.sync if si < 2 else nc.scalar
            s = 4 * g + si
            eng.dma_start(
                out=lt[32 * si : 32 * si + 32, :, :, 0:D].rearrange(
                    "nl win wv d -> nl (win wv) d"
                ),
                in_=x[s].rearrange("(wvs nl) d -> nl wvs d", nl=32),
            )
        lt2 = ld.tile([P, NWIN, 8, 32], F32, name="ldt2", tag="ldt2")
        ceng = nc.scalar if g == 0 else nc.gpsimd
        for si in range(4):
            # staging copies: each waits on exactly one DMA queue semaphore
            cop = ceng.copy if g == 0 else ceng.tensor_copy
            cop(
                out=lt2[32 * si : 32 * si + 32, :, :, 0:D].rearrange(
                    "nl win wv d -> nl (win wv) d"
                ),
                in_=lt[32 * si : 32 * si + 32, :, :, 0:D].rearrange(
                    "nl win wv d -> nl (win wv) d"
                ),
            )
        nc.vector.tensor_copy(out=lt2[:, :, :, D:32], in_=lt2[:, :, :, 0:D])
        for win in range(NWIN):
            # block transpose: out[32si+d, 32wv+nl] = lt2[32si+nl, win, wv, d]
            nc.vector.transpose(
                out=xt4[g][:, 256 * win : 256 * (win + 1)],
                in_=lt2[:, win, :, :].rearrange("p wv d -> p (wv d)"),
            )

    # bf16 hi/lo packed xT4: rows 32si+d hold -x_hi, rows 32si+16+d hold
    # x_lo.  With QA rows (-Qhi | +Qhi) and QB rows (-Qlo | 0):
    # b = xHL@QA + xHL@QB = xhi@Qhi + xlo@Qhi + xhi@Qlo   (~1e-5 rel)
    maskLO = const.tile([P, 1], F32)
    sgn = const.tile([P, 1], F32)
    posz = const.tile([P, 1], F32)
    nc.vector.memset(maskLO, 1.0)
    nc.vector.memset(sgn, 1.0)
    nc.vector.memset(posz, 0.0)
    for si in range(4):
        nc.vector.memset(maskLO[32 * si : 32 * si + D, :], 0.0)
        nc.vector.memset(sgn[32 * si : 32 * si + D, :], -1.0)
        nc.vector.memset(posz[32 * si : 32 * si + D, :], 1.0)
    xt4hl = [
        const.tile([P, N], BF16, name=f"xt4hl_{g}", tag=f"xt4hl_{g}")
        for g in range(G)
    ]
    for g in range(G):
        xthb = small.tile([P, N], BF16, tag="xthb")
        H = N // 2
        for hh in range(2):
            sl = slice(hh * H, (hh + 1) * H)
            nc.vector.tensor_copy(out=xthb[:, sl], in_=xt4[g][:, sl])
            nc.vector.scalar_tensor_tensor(
                out=xt4hl[g][:, sl], in0=xt4[g][:, sl], scalar=maskLO,
                in1=xthb[:, sl], op0=ALU.mult, op1=ALU.subtract,
            )

    # Q_bd[g]: [128, 128]; rows 32s'+d (d<16) hold Q_s^T, cols 32s'+m
    qbd = [const.tile([P, P], F32, name=f"qbd_{g}", tag=f"qbd_{g}") for g in range(G)]
    qa = [const.tile([P, P], BF16, name=f"qa_{g}", tag=f"qa_{g}") for g in range(G)]
    qb = [const.tile([P, P], BF16, name=f"qb_{g}", tag=f"qb_{g}") for g in range(G)]
    for g in range(G):
        nc.vector.memset(qbd[g], 0.0)

    # ---------------- routing iterations ----------------
    for r in range(1, ROUTINGS + 1):
        c_sb = {}
        for g in range(G):
            if r > 1:
                # logits b = x @ Q_{r-1}, then c_unnorm = exp(b)
                # c_sb[g][p, k, 32s'+m] = exp(b)[128k+p, (s',m)]
                c_sb[g] = work.tile([P, NCHUNK, P], F32, name=f"c_{g}", tag=f"c_{g}")
                for half in range(NCHUNK // 8):  # 2 psum tiles of 8 chunks
                    bp = psum_b.tile([P, 8, P], F32, tag="bpsum")
                    # absorber: zero-writing opener takes the PE drain wait
                    nc.tensor.matmul(out=bp[:, 0, :], lhsT=zb[:, :],
                                     rhs=zb[:, :], start=True, stop=False)
                    for i in range(8):
                        k = 8 * half + i
                        for j, rh in enumerate((qa[g], qb[g])):
                            nc.tensor.matmul(
                                out=bp[:, i, :],
                                lhsT=xt4hl[g][:, 128 * k : 128 * (k + 1)],
                                rhs=rh[:, :],
                                start=(j == 0 and i > 0),
                                stop=(j == 1),
                            )
                    nc.scalar.activation(
                        out=c_sb[g][:, 8 * half : 8 * half + 8, :].rearrange(
                            "p k f -> p (k f)"
                        ),
                        in_=bp[:, :, :].rearrange("p k f -> p (k f)"),
                        func=ACTF.Exp,
                    )

            # ---- T[m,d] and Z accumulated over n-chunks on PE ----
            # one accumulation group per bank: lhsT = 4 samples' c blocks,
            # rhs = the same 4 samples' [x|1] quads; diagonal blocks extracted
            tz = psum_t.tile([P, 72], F32, tag="tz")
            # absorber A: takes the PE psum-slot drain wait (opens group).
            # lhsT=qlo delays its readiness past the logits matmuls so the
            # scheduler doesn't hoist it before the DVE tick is absorbed.
            a_lhs = qb[g] if r > 1 else zb
            nc.tensor.matmul(out=tz[:, 68:70], lhsT=a_lhs[:, :],
                             rhs=zz[:, :], start=True, stop=False)
            if r > 1:
                # absorber B: takes the ACT (exp) wait
                cb = c_sb[g][:, 0, 0:2]
                nc.tensor.matmul(out=tz[0:2, 70:72], lhsT=cb, rhs=cb,
                                 start=False, stop=False)
            for k in range(NCHUNK):
                lhsT = cconst[:, :] if r == 1 else c_sb[g][:, k, :]
                nc.tensor.matmul(
                    out=tz[:, 0:68],
                    lhsT=lhsT,
                    rhs=xq[g][:, k, :, :].rearrange("p s f -> p (s f)"),
                    start=False,
                    stop=(k == NCHUNK - 1),
                )

            t4 = small.tile([P, D + 1], F32, tag="t4")
            for si in range(4):
                nc.vector.tensor_copy(
                    out=t4[32 * si : 32 * si + 32, :],
                    in_=tz[32 * si : 32 * si + 32, 17 * si : 17 * si + 17],
                )
            rz = small.tile([P, 1], F32, tag="rz")
            nc.vector.reciprocal(out=rz, in_=t4[:, D : D + 1])

            # ---- s[m,c] = (sum_d T[m,d] W[d,m,c]) / Z ----
            prod = small.tile([P, D, C], F32, tag="prod")
            nc.vector.tensor_tensor(
                out=prod[:, :, :],
                in0=_bcast(t4[:, 0:D], 1, C),
                in1=w4[:, :, :],
                op=ALU.mult,
            )
            s4 = small.tile([P, C], F32, tag="s4")
            nc.vector.tensor_reduce(
                out=s4[:, :],
                in_=prod[:, :, :].rearrange("p d c -> p c d"),
                axis=mybir.AxisListType.X,
                op=ALU.add,
            )
            nc.vector.tensor_scalar_mul(out=s4[:, :], in0=s4[:, :], scalar1=rz)

            # ---- squash ----
            n2 = small.tile([P, 1], F32, tag="n2")
            sq = small.tile([P, C], F32, tag="sq")
            nc.vector.scalar_tensor_tensor(
                out=sq[:, :],
                in0=s4[:, :],
                scalar=1.0,
                in1=s4[:, :],
                op0=ALU.mult,
                op1=ALU.mult,
                accum_out=n2,
            )
            # y ~= rsqrt(n2) : magic seed + 3 Newton steps (no ACT table switch)
            y = small.tile([P, 1], F32, tag="y")
            hlf = small.tile([P, 1], F32, tag="hlf")
            nc.vector.tensor_scalar(
                out=y.bitcast(mybir.dt.int32),
                in0=n2.bitcast(mybir.dt.int32),
                scalar1=1,
                scalar2=None,
                op0=ALU.logical_shift_right,
            )
            nc.vector.tensor_scalar(
                out=y.bitcast(mybir.dt.int32),
                in0=y.bitcast(mybir.dt.int32),
                scalar1=-1,
                scalar2=RSQRT_MAGIC,
                op0=ALU.mult,
                op1=ALU.add,
            )
            for _ in range(2):
                nc.vector.tensor_mul(out=hlf, in0=y, in1=y)
                nc.vector.tensor_mul(out=hlf, in0=hlf, in1=n2)
                nc.vector.tensor_scalar(
                    out=hlf,
                    in0=hlf,
                    scalar1=-0.5,
                    scalar2=1.5,
                    op0=ALU.mult,
                    op1=ALU.add,
                )
                nc.vector.tensor_mul(out=y, in0=y, in1=hlf)
            nrm = small.tile([P, 1], F32, tag="nrm")
            nc.vector.tensor_mul(out=nrm, in0=n2, in1=y)  # = sqrt(n2)
            one_p_n2 = small.tile([P, 1], F32, tag="opn")
            nc.vector.tensor_scalar_add(out=one_p_n2, in0=n2, scalar1=1.0)
            nc.vector.tensor_scalar_add(out=nrm, in0=nrm, scalar1=EPS)
            den = small.tile([P, 1], F32, tag="den")
            nc.vector.tensor_mul(out=den, in0=one_p_n2, in1=nrm)
            nc.vector.reciprocal(out=den, in_=den)
            fct = small.tile([P, 1], F32, tag="fct")
            nc.vector.tensor_mul(out=fct, in0=n2, in1=den)
            v4 = small.tile([P, C], F32, tag="v4")
            nc.vector.tensor_scalar_mul(out=v4[:, :], in0=s4[:, :], scalar1=fct)

            if r == ROUTINGS:
                vstage = small.tile([P, C], F32, tag="vstage")
                nc.gpsimd.tensor_copy(out=vstage[:, :], in_=v4[:, :])
                nc.gpsimd.dma_start(
                    out=vout[4 * g : 4 * g + 4].rearrange("s m c -> (s m) c"),
                    in_=vstage[:, :],
                )
                continue

            # ---- P[m,d] = sum_c v[m,c] W[d,m,c]; Q += P^T (block diag) ----
            prod2 = small.tile([P, D, C], F32, tag="prod2")
            nc.vector.tensor_tensor(
                out=prod2[:, :, :],
                in0=_bcast(v4[:, :], 0, D),
                in1=w4[:, :, :],
                op=ALU.mult,
            )
            p4 = small.tile([P, 32], F32, tag="p4")
            nc.vector.tensor_reduce(
                out=p4[:, 0:D],
                in_=prod2[:, :, :],
                axis=mybir.AxisListType.X,
                op=ALU.add,
            )
            nc.vector.tensor_copy(out=p4[:, D:32], in_=p4[:, 0:D])
            # DVE 32x32 block transpose: p4t[32si+d, m] = p4[32si+m, d]
            p4t = small.tile([P, 32], F32, tag="p4t")
            nc.vector.transpose(out=p4t[:, :], in_=p4[:, :])
            for si in range(4):
                srcb = p4t[32 * si : 32 * si + 32, 0:32]
                dst = qbd[g][32 * si : 32 * si + 32, 32 * si : 32 * si + 32]
                if r == 1:
                    nc.vector.tensor_copy(out=dst, in_=srcb)
                else:
                    nc.vector.tensor_add(out=dst, in0=dst, in1=srcb)
            # QA = bf16(qbd)*sgn; QB = bf16(qh - qbd) masked to hi rows
            nc.vector.tensor_scalar_mul(out=qa[g][:, :], in0=qbd[g][:, :],
                                        scalar1=sgn)
            mlq = small.tile([P, P], F32, tag="mlq")
            nc.vector.scalar_tensor_tensor(
                out=mlq[:, :], in0=qa[g][:, :], scalar=sgn,
                in1=qbd[g][:, :], op0=ALU.mult, op1=ALU.subtract,
            )
            nc.vector.tensor_scalar_mul(out=qb[g][:, :], in0=mlq[:, :],
                                        scalar1=posz)

    ctx.close()


_NC_CACHE = None
_RUNNER = None


def _get_nc():
    global _NC_CACHE
    if _NC_CACHE is None:
        _NC_CACHE = build_bass()
    return _NC_CACHE


def _get_runner():
    """Build the sharded jitted executable once and reuse it across calls
    (run_bass_kernel_spmd re-traces jax on every invocation)."""
    global _RUNNER
    if _RUNNER is not None:
        return _RUNNER
    import jax
    import jax.numpy as jnp
    from jax.sharding import Mesh, PartitionSpec
    from jax.experimental.shard_map import shard_map
    from concourse import bass2jax, mybir as mb
    from concourse.bass2jax import (
        _bass_exec_p,
        install_neuronx_cc_hook,
        partition_id_tensor,
    )

    install_neuronx_cc_hook()
    nc = _get_nc()

    part_name = nc.partition_id_tensor.name if nc.partition_id_tensor else None
    in_names, out_names, out_avals, zero_outs = [], [], [], []
    for alloc in nc.m.functions[0].allocations:
        if not isinstance(alloc, mb.MemoryLocationSet):
            continue
        name = alloc.memorylocations[0].name
        if alloc.kind == "ExternalInput":
            if name != part_name:
                in_names.append(name)
        elif alloc.kind == "ExternalOutput":
            out_names.append(name)
            shape = tuple(alloc.tensor_shape)
            dtype = mb.dt.np(alloc.dtype)
            out_avals.append(jax.core.ShapedArray(shape, dtype))
            zero_outs.append(np.zeros(shape, dtype))
    n_params = len(in_names)
    all_names = in_names + out_names
    if part_name is not None:
        all_names.append(part_name)

    def _body(*args):
        operands = list(args)
        if part_name is not None:
            operands.append(partition_id_tensor())
        outs = _bass_exec_p.bind(
            *operands,
            out_avals=tuple(out_avals),
            in_names=tuple(all_names),
            out_names=tuple(out_names),
            lowering_input_output_aliases=(),
            sim_require_finite=True,
            sim_require_nnan=True,
            nc=nc,
        )
        return tuple(outs)

    devices = jax.devices()[:NCORES]
    mesh = Mesh(np.asarray(devices), ("core",))
    n_outs = len(out_names)
    sharded = jax.jit(
        shard_map(
            _body,
            mesh=mesh,
            in_specs=(PartitionSpec("core"),) * (n_params + n_outs),
            out_specs=(PartitionSpec("core"),) * n_outs,
            check_rep=False,
        ),
        donate_argnums=tuple(range(n_params, n_params + n_outs)),
        keep_unused=True,
    )
    _RUNNER = (sharded, in_names, zero_outs)
    return _RUNNER


def kernel(inputs: np.ndarray, W: np.ndarray) -> np.ndarray:
    inputs = np.ascontiguousarray(np.asarray(inputs, dtype=np.float32))
    W = np.ascontiguousarray(np.asarray(W, dtype=np.float32))
    sharded, in_names, zero_outs = _get_runner()
    per_name = {
        "x": inputs.reshape(NCORES * BL, N, D),
        "w": np.concatenate([W] * NCORES, axis=0),
    }
    concat_in = [per_name[n] for n in in_names]
    concat_zeros = [
        np.zeros((NCORES * z.shape[0], *z.shape[1:]), z.dtype) for z in zero_outs
    ]
    out_arrs = sharded(*concat_in, *concat_zeros)
    return np.asarray(out_arrs[0]).reshape(B, M, C).astype(np.float32)



# revision 29
# speedup vs baseline: 1.4079x; 1.4079x over previous
"""CapsuleLayer dynamic-routing kernel for 8 Trainium2 NeuronCores.

Math (per sample):
    u_hat[n,m,c] = sum_d x[n,d] W[d,m,c]           (never materialized)
    routing r=1..3:
        c = softmax_n(b)            -> c_unnorm = exp(b), Z[m] = sum_n c_unnorm
        s[m,c] = sum_n c[n,m] u_hat[n,m,c] = (sum_d T[m,d] W[d,m,c]) / Z[m]
                 where T[m,d] = sum_n c_unnorm[n,m] x[n,d]
        v = squash(s)
        b += sum_c v[m,c] u_hat[n,m,c] = x @ P_r.T  where P_r[m,d] = sum_c v W
    With Q_r = sum_{r'<=r} P_r', the logits are always b_r = x @ Q_r.T, so we
    accumulate Q (tiny) instead of b (big).  exp() overflow-safe without the
    max-subtraction: |b| <~ 60 << 88.

Layout: n = 16*p + q (p partition, q chunk) so the x load is one DMA of
1KB-contiguous runs.  x^T (for the logits matmul) is built on-chip via PE
transposes of the bf16 hi/lo-packed x; T/Z matmuls run in bf16 with
per-sample diagonal psum placement; squash's rsqrt uses ACT ln/exp (same
activation table as the softmax exp, so no table switches).

Sharding: batch 64 -> 8 samples/core, fully independent.
"""

import os
import sys

import numpy as np

for _p in ("/opt/trn_rl_repo", os.path.expanduser("~/.axon_site/_ro/trn_rl_repo")):
    if os.path.isdir(_p) and _p not in sys.path:
        sys.path.insert(0, _p)

import concourse.bass as bass
import concourse.tile as tile
from concourse import masks, mybir
from concourse.vector_clock import ScopedClock, VectorClock
from bass_rust import N_PROCS


class _SplitDrainTC(tile.TileContext):
    """TileContext whose exit drain is split into several drains with few
    sem waits each: walrus rejects a single drain waiting on >~8 sems."""

    def _drain_and_barrier(self, tick_clock, wait_clock):
        gc = tick_clock.global_clock
        CH = 1
        for i in range(0, N_PROCS, CH):
            sub = VectorClock(
                [gc[p] if i <= p < i + CH else 0 for p in range(N_PROCS)]
            )
            drain_inst = self.nc.sync.drain()
            wait_clock.add_sem_waits(
                drain_inst.ins, ScopedClock({None: sub})
            )
        self.nc.all_engine_barrier()
        assert self.sems is not None
        popped = self.nc._tile_sem_poison_stack.pop()
        assert popped is self._sem_poison
        self.nc.clear_and_free_semaphores(list(self.sems.allocated().values()))
        self.nc.all_engine_barrier()

B, N, D, M, C = 64, 2048, 16, 32, 16
NCORES = 8
BL = B // NCORES          # samples per core = 8
G = BL // 4               # sample groups of 4 -> 2
NQ = N // 128             # 16 chunks; n = 16*p + q
ROUTINGS = 3
F32 = mybir.dt.float32
BF16 = mybir.dt.bfloat16
ALU = mybir.AluOpType
ACTF = mybir.ActivationFunctionType


def _bcast(ap, idx, num):
    """Insert a stride-0 free dim of size `num` at free-dim position idx."""
    dims = list(ap.ap)
    dims.insert(1 + idx, [0, num])
    return bass.AP(ap.tensor, ap.offset, dims)


def build_bass():
    nc = bass.Bass()
    x_in = nc.declare_dram_parameter("x", [BL, N, D], F32, isOutput=False)
    w_in = nc.declare_dram_parameter("w", [D, M, C], F32, isOutput=False)
    v_out = nc.declare_dram_parameter("v", [BL, M, C], F32, isOutput=True)

    with _SplitDrainTC(nc) as tc:
        _emit(tc, x_in, w_in, v_out)
    return nc


def _emit(tc, x_in, w_in, v_out):
    nc = tc.nc
    P = 128

    from contextlib import ExitStack

    ctx = ExitStack()
    const = ctx.enter_context(tc.tile_pool(name="const", bufs=1))
    work = ctx.enter_context(tc.tile_pool(name="work", bufs=2))
    small = ctx.enter_context(tc.tile_pool(name="small", bufs=4))
    psum_b = ctx.enter_context(tc.tile_pool(name="psum_b", bufs=2, space="PSUM"))
    psum_t = ctx.enter_context(tc.tile_pool(name="psum_t", bufs=2, space="PSUM"))

    x = x_in[:]
    w = w_in[:]
    vout = v_out[:]

    # ---------------- input staging ----------------
    # xq_raw[g][p, s, q, d] = x[4g+s, 16p+q, d]: one DMA per group, runs of
    # 1KB ((q d) contiguous on HBM for fixed (s, p)).
    xq_raw = []
    for g in range(G):
        t = const.tile([P, 4, NQ, D], F32, name=f"xq_raw_{g}", tag=f"xq_raw_{g}")
        eng = nc.sync if g == 0 else nc.scalar
        eng.dma_start(
            out=t, in_=x[4 * g : 4 * g + 4].rearrange("s (p q) d -> p s q d", p=P)
        )
        xq_raw.append(t)

    # identity for PE transposes (emitted first so its Pool tick is
    # subsumed by the later hi-copy ticks)
    ident = const.tile([P, P], BF16, name="ident", tag="ident")
    masks.make_identity(nc, ident[:, :])
    # ACT-written zeros: the bp opener's operands, so its only sem wait
    # (the ACT exp pool-recycle) coalesces with the operand dep
    zact = const.tile([P, P], BF16, name="zact", tag="zact")
    nc.scalar.mul(out=zact[:, :], in_=ident[:, :], mul=0.0)
    # DVE-written zeros for the tz opener (coalesces with its DVE waits)
    zb = const.tile([P, P], BF16)
    nc.vector.memset(zb, 0.0)
    # DVE-written identity copy for the first staging absorber
    identD = const.tile([P, P], BF16, name="identD", tag="identD")
    nc.vector.tensor_copy(out=identD, in_=ident[:, :])
    # r=1 uniform softmax weights (also the Pool-written operand of the
    # second staging absorber); emitted before the hi copies so its Pool
    # tick is subsumed by theirs
    cconst = const.tile([P, P], BF16)
    nc.gpsimd.memset(cconst, 1.0 / N)

    # Two bf16 staging tiles (q-major so per-chunk matmul operands merge
    # to 2D APs for walrus):
    #   xthl[g][p, q, s, 0:16] = hi(x), [.., 16:32] = lo   (transpose source)
    #   xqz[g][p, q, s, 0] = 1.0, [.., 1:17] = hi(x)       (T/Z rhs)
    # hi on Pool (its wait clock subsumes the DMA for all consumers),
    # lo on DVE.
    xthl, xqz = [], []
    for g in range(G):
        th = const.tile([P, NQ, 4, 32], BF16, name=f"xthl_{g}", tag=f"xthl_{g}")
        tz_ = const.tile([P, NQ, 4, 17], BF16, name=f"xqz_{g}", tag=f"xqz_{g}")
        nc.gpsimd.memset(tz_[:, :, :, 0:1], 1.0)
        for si in range(4):
            nc.gpsimd.tensor_copy(
                out=th[:, :, si, 0:16], in_=xq_raw[g][:, si, :, :]
            )
            nc.gpsimd.tensor_copy(
                out=tz_[:, :, si, 1:17], in_=th[:, :, si, 0:16]
            )
        # 2D absorber takes the Pool+DMA waits; it produces the -1.0
        # scalar ptr the 3D lo STTs read, pinning them after it in DVE
        # order so they carry no sem waits (walrus rejects 3D
        # instructions with >1 wait).
        j1 = const.tile([P, 2], F32, name=f"j1_{g}", tag=f"j1_{g}")
        j2 = const.tile([P, 2], F32, name=f"j2_{g}", tag=f"j2_{g}")
        nc.vector.tensor_copy(out=j1, in_=th[:, NQ - 1, 3, 0:2])
        nc.vector.tensor_copy(out=j2, in_=xq_raw[g][:, 3, NQ - 1, 0:2])
        zt = const.tile([P, 2], F32, name=f"zt_{g}", tag=f"zt_{g}")
        nc.vector.tensor_tensor(out=zt, in0=j1, in1=j2, op=ALU.mult)
        negone = const.tile([P, 1], F32, name=f"negone_{g}", tag=f"negone_{g}")
        nc.vector.tensor_scalar(
            out=negone, in0=zt[:, 0:1], scalar1=0.0, scalar2=-1.0,
            op0=ALU.mult, op1=ALU.add,
        )
        for si in range(4):
            nc.vector.scalar_tensor_tensor(
                out=th[:, :, si, 16:32], in0=th[:, :, si, 0:16],
                scalar=negone, in1=xq_raw[g][:, si, :, :],
                op0=ALU.mult, op1=ALU.add,
            )
        xthl.append(th)
        xqz.append(tz_)

    # xt4hl[g][32s+16hl+dd, q, p] = xqhl[g][p, s, q, 1+16hl+dd]
    # (rows: hi block then lo block per sample; cols of chunk q = partition p)
    xt4hl = [
        const.tile([P, NQ, P], BF16, name=f"xt4hl_{g}", tag=f"xt4hl_{g}")
        for g in range(G)
    ]
    prev_copy = None
    for g in range(G):
        for h in range(2):
            tp = psum_b.tile([P, 8, P], BF16, tag="tp")
            # chained single-wait PE absorbers: #0 takes the psum-slot
            # drain wait (its DVE recycle tick was already waited by the
            # previous block's #1), #1 waits the latest DVE tick (most
            # recent xt4hl copy, subsuming the lo stts and any tp-buffer
            # reader), #2 the Pool (hi copy) wait; the 8 transposes then
            # carry none.
            nc.tensor.matmul(out=tp[0:2, 0, :],
                             lhsT=identD[:, 0:2],
                             rhs=identD[:, :], is_transpose=True,
                             skip_group_check=True)
            dve_src = (
                xthl[g][:, 0, 3, 16:18] if prev_copy is None
                else prev_copy[:, 0:2]
            )
            nc.tensor.matmul(out=tp[0:2, 0, :],
                             lhsT=dve_src,
                             rhs=identD[:, :], is_transpose=True,
                             skip_group_check=True)
            nc.tensor.matmul(out=tp[0:2, 0, :],
                             lhsT=xthl[g][:, 0, 3, 0:2],
                             rhs=ident[:, :], is_transpose=True,
                             skip_group_check=True)
            for i in range(8):
                q = 8 * h + i
                nc.tensor.transpose(
                    out=tp[:, i, :],
                    in_=xthl[g][:, q, :, :],
                    identity=ident[:, :],
                )
            nc.vector.tensor_copy(
                out=xt4hl[g][:, 8 * h : 8 * h + 8, :].rearrange(
                    "p k f -> p (k f)"
                ),
                in_=tp[:, :, :].rearrange("p k f -> p (k f)"),
            )
            prev_copy = xt4hl[g][:, 8 * h, :]

    # w4 = W replicated across the 4 samples, via one DMA + Pool copies
    # (so every later consumer carries a single Pool wait)
    w1 = const.tile([32, D, C], F32)
    nc.scalar.dma_start(out=w1, in_=w.rearrange("d m c -> m d c"))
    w4 = const.tile([P, D, C], F32)
    for gi in range(4):
        nc.gpsimd.tensor_copy(out=w4[32 * gi : 32 * gi + 32, :, :], in_=w1)

    # maskHI: 1.0 on hi rows (row%32 < 16), 0.0 on lo rows
    maskHI = const.tile([P, 1], F32)
    nc.vector.memset(maskHI, 0.0)
    for si in range(4):
        nc.vector.memset(maskHI[32 * si : 32 * si + 16, :], 1.0)

    # Q storage (compact, fp32) and the bf16 logits rhs matrices A/B
    # (block-diagonal: A rows (s,hl,d) cols (s,m) hold Qhi[m,d] on both hl
    # blocks; B holds Qlo[m,d] on hi rows, 0 on lo rows)
    qbd = [const.tile([P, 32], F32, name=f"qbd_{g}", tag=f"qbd_{g}") for g in range(G)]
    qa = [const.tile([P, P], BF16, name=f"qa_{g}", tag=f"qa_{g}") for g in range(G)]
    qb = [const.tile([P, P], BF16, name=f"qb_{g}", tag=f"qb_{g}") for g in range(G)]
    for g in range(G):
        nc.vector.memset(qa[g], 0.0)
        nc.vector.memset(qb[g], 0.0)

    # ---------------- routing iterations ----------------
    for r in range(1, ROUTINGS + 1):
        c_sb = {}
        for g in range(G):
            if r > 1:
                # logits b = x @ Q_{r-1}, then c_unnorm = exp(b) in bf16
                c_sb[g] = work.tile([P, NQ, P], BF16, name=f"c_{g}", tag=f"c_{g}")
                for half in range(2):
                    bp = psum_b.tile([P, 8, P], F32, tag="bpsum")
                    # absorber: zero-writing opener takes the pool drain wait
                    nc.tensor.matmul(out=bp[:, 0, :], lhsT=zact[:, :],
                                     rhs=zact[:, :], start=True, stop=False)
                    for i in range(8):
                        q = 8 * half + i
                        for j, rh in enumerate((qa[g], qb[g])):
                            nc.tensor.matmul(
                                out=bp[:, i, :],
                                lhsT=xt4hl[g][:, q, :],
                                rhs=rh[:, :],
                                start=(j == 0 and i > 0),
                                stop=(j == 1),
                            )
                    nc.scalar.activation(
                        out=c_sb[g][:, 8 * half : 8 * half + 8, :].rearrange(
                            "p k f -> p (k f)"
                        ),
                        in_=bp[:, :, :].rearrange("p k f -> p (k f)"),
                        func=ACTF.Exp,
                    )

            # ---- T[m,d] and Z accumulated over n-chunks on PE (bf16) ----
            # full-width: out[(s,m), 17s'+j] block-diagonal; per chunk the
            # [ones|hi] rhs writes cols 0:17 per sample, then the lo rhs
            # accumulates into the T columns via a strided 3D out AP.
            tz = psum_t.tile([P, 72], F32, tag="tz")
            tz4 = tz[:, 0:68].rearrange("p (s f) -> p s f", s=4)
            # absorber A: takes the psum-slot drain wait (opens group) plus
            # the Pool (zb memset, emitted after ones+hi so its tick subsumes
            # them) and DVE (lo) waits for xqhl.
            a_lhs = qb[g] if r > 1 else zb
            nc.tensor.matmul(out=tz[:, 68:70], lhsT=a_lhs[:, :],
                             rhs=xthl[g][:, 0, 0, 16:18], start=True,
                             stop=False)
            if r > 1:
                # absorber B: takes the ACT (exp) wait
                cb = c_sb[g][:, 0, 0:2]
                nc.tensor.matmul(out=tz[0:2, 70:72], lhsT=cb, rhs=cb,
                                 start=False, stop=False)
            for q in range(NQ):
                lhsT = cconst[:, :] if r == 1 else c_sb[g][:, q, :]
                nc.tensor.matmul(
                    out=tz[:, 0:68],
                    lhsT=lhsT,
                    rhs=xqz[g][:, q, :, :],
                    start=False,
                    stop=(q == NQ - 1),
                )

            # ---- s[m,c] = (sum_d T[m,d] W[d,m,c]) / Z ----
            # extract the diagonal sample blocks: t4[(s,m), 0]=Z, [.,1:17]=T
            t4 = small.tile([P, 17], F32, tag="t4")
            for si in range(4):
                nc.vector.tensor_copy(
                    out=t4[32 * si : 32 * si + 32, :],
                    in_=tz4[32 * si : 32 * si + 32, si, :],
                )
            rz = small.tile([P, 1], F32, tag="rz")
            nc.vector.reciprocal(out=rz, in_=t4[:, 0:1])
            prod = small.tile([P, D, C], F32, tag="prod")
            nc.vector.tensor_tensor(
                out=prod[:, :, :],
                in0=_bcast(t4[:, 1:17], 1, C),
                in1=w4[:, :, :],
                op=ALU.mult,
            )
            s4 = small.tile([P, C], F32, tag="s4")
            nc.vector.tensor_reduce(
                out=s4[:, :],
                in_=prod[:, :, :].rearrange("p d c -> p c d"),
                axis=mybir.AxisListType.X,
                op=ALU.add,
            )
            nc.vector.tensor_scalar_mul(out=s4[:, :], in0=s4[:, :], scalar1=rz)

            # ---- squash: v = s * n/(1+n^2), n = |s|; rsqrt via ACT ln/exp
            n2 = small.tile([P, 1], F32, tag="n2")
            sq = small.tile([P, C], F32, tag="sq")
            nc.vector.scalar_tensor_tensor(
                out=sq[:, :],
                in0=s4[:, :],
                scalar=1.0,
                in1=s4[:, :],
                op0=ALU.mult,
                op1=ALU.mult,
                accum_out=n2,
            )
            # fct = n/(1+n^2) = exp(0.5 ln n2 - ln(1+n2)); Ln and Exp share
            # one ACT table, and no instruction reads a ptr written by its
            # own engine (which would force an extra self sem wait)
            l1 = small.tile([P, 1], F32, tag=f"l1_{g}")
            nc.scalar.activation(out=l1, in_=n2, func=ACTF.Ln)
            l2 = small.tile([P, 1], F32, tag=f"l2_{g}")
            nc.scalar.activation(out=l2, in_=n2, func=ACTF.Ln, bias=1.0)
            comb = small.tile([P, 1], F32, tag="comb")
            nc.vector.scalar_tensor_tensor(
                out=comb, in0=l1, scalar=0.5, in1=l2,
                op0=ALU.mult, op1=ALU.subtract,
            )
            fct = small.tile([P, 1], F32, tag=f"fct_{g}")
            nc.scalar.activation(out=fct, in_=comb, func=ACTF.Exp)
            v4 = small.tile([P, C], F32, tag="v4")
            nc.vector.tensor_scalar_mul(out=v4[:, :], in0=s4[:, :], scalar1=fct)

            if r == ROUTINGS:
                vstage = small.tile([P, C], F32, tag="vstage")
                nc.gpsimd.tensor_copy(out=vstage[:, :], in_=v4[:, :])
                nc.gpsimd.dma_start(
                    out=vout[4 * g : 4 * g + 4].rearrange("s m c -> (s m) c"),
                    in_=vstage[:, :],
                )
                continue

            # ---- P[m,d] = sum_c v[m,c] W[d,m,c]; Q += P^T; rebuild A/B ----
            prod2 = small.tile([P, D, C], F32, tag="prod2")
            nc.vector.tensor_tensor(
                out=prod2[:, :, :],
                in0=_bcast(v4[:, :], 0, D),
                in1=w4[:, :, :],
                op=ALU.mult,
            )
            p4 = small.tile([P, 32], F32, tag="p4")
            nc.vector.tensor_reduce(
                out=p4[:, 0:D],
                in_=prod2[:, :, :],
                axis=mybir.AxisListType.X,
                op=ALU.add,
            )
            nc.vector.tensor_copy(out=p4[:, D:32], in_=p4[:, 0:D])
            # DVE 32x32 block transpose: p4t[32s+16hl+d, m] = P[m, d]
            p4t = small.tile([P, 32], F32, tag="p4t")
            nc.vector.transpose(out=p4t[:, :], in_=p4[:, :])
            if r == 1:
                nc.vector.tensor_copy(out=qbd[g][:, :], in_=p4t[:, :])
            else:
                nc.vector.tensor_add(out=qbd[g][:, :], in0=qbd[g][:, :],
                                     in1=p4t[:, :])
            # compact Qhi/Qlo, then place on the diagonal blocks of A/B
            acomp = small.tile([P, 32], BF16, tag="acomp")
            nc.vector.tensor_copy(out=acomp[:, :], in_=qbd[g][:, :])
            mlq = small.tile([P, 32], F32, tag="mlq")
            nc.vector.scalar_tensor_tensor(
                out=mlq[:, :], in0=acomp[:, :], scalar=-1.0,
                in1=qbd[g][:, :], op0=ALU.mult, op1=ALU.add,
            )
            bcomp = small.tile([P, 32], BF16, tag="bcomp")
            nc.vector.tensor_scalar_mul(out=bcomp[:, :], in0=mlq[:, :],
                                        scalar1=maskHI)
            for si in range(4):
                sl = slice(32 * si, 32 * si + 32)
                nc.vector.tensor_copy(out=qa[g][sl, sl], in_=acomp[sl, :])
                nc.vector.tensor_copy(out=qb[g][sl, sl], in_=bcomp[sl, :])

    ctx.close()


_NC_CACHE = None
_RUNNER = None


def _get_nc():
    global _NC_CACHE
    if _NC_CACHE is None:
        _NC_CACHE = build_bass()
    return _NC_CACHE


def _get_runner():
    """Build the sharded jitted executable once and reuse it across calls
    (run_bass_kernel_spmd re-traces jax on every invocation)."""
    global _RUNNER
    if _RUNNER is not None:
        return _RUNNER
    import jax
    import jax.numpy as jnp
    from jax.sharding import Mesh, PartitionSpec
    from jax.experimental.shard_map import shard_map
    from concourse import bass2jax, mybir as mb
    from concourse.bass2jax import (
        _bass_exec_p,
        install_neuronx_cc_hook,
        partition_id_tensor,
    )

    install_neuronx_cc_hook()
    nc = _get_nc()

    part_name = nc.partition_id_tensor.name if nc.partition_id_tensor else None
    in_names, out_names, out_avals, zero_outs = [], [], [], []
    for alloc in nc.m.functions[0].allocations:
        if not isinstance(alloc, mb.MemoryLocationSet):
            continue
        name = alloc.memorylocations[0].name
        if alloc.kind == "ExternalInput":
            if name != part_name:
                in_names.append(name)
        elif alloc.kind == "ExternalOutput":
            out_names.append(name)
            shape = tuple(alloc.tensor_shape)
            dtype = mb.dt.np(alloc.dtype)
            out_avals.append(jax.core.ShapedArray(shape, dtype))
            zero_outs.append(np.zeros(shape, dtype))
    n_params = len(in_names)
    all_names = in_names + out_names
    if part_name is not None:
        all_names.append(part_name)

    def _body(*args):
        operands = list(args)
        if part_name is not None:
            operands.append(partition_id_tensor())
        outs = _bass_exec_p.bind(
            *operands,
            out_avals=tuple(out_avals),
            in_names=tuple(all_names),
            out_names=tuple(out_names),
            lowering_input_output_aliases=(),
            sim_require_finite=True,
            sim_require_nnan=True,
            nc=nc,
        )
        return tuple(outs)

    devices = jax.devices()[:NCORES]
    mesh = Mesh(np.asarray(devices), ("core",))
    n_outs = len(out_names)
    sharded = jax.jit(
        shard_map(
            _body,
            mesh=mesh,
            in_specs=(PartitionSpec("core"),) * (n_params + n_outs),
            out_specs=(PartitionSpec("core"),) * n_outs,
            check_rep=False,
        ),
        donate_argnums=tuple(range(n_params, n_params + n_outs)),
        keep_unused=True,
    )
    _RUNNER = (sharded, in_names, zero_outs)
    return _RUNNER


def kernel(inputs: np.ndarray, W: np.ndarray) -> np.ndarray:
    inputs = np.ascontiguousarray(np.asarray(inputs, dtype=np.float32))
    W = np.ascontiguousarray(np.asarray(W, dtype=np.float32))
    sharded, in_names, zero_outs = _get_runner()
    per_name = {
        "x": inputs.reshape(NCORES * BL, N, D),
        "w": np.concatenate([W] * NCORES, axis=0),
    }
    concat_in = [per_name[n] for n in in_names]
    concat_zeros = [
        np.zeros((NCORES * z.shape[0], *z.shape[1:]), z.dtype) for z in zero_outs
    ]
    out_arrs = sharded(*concat_in, *concat_zeros)
    return np.asarray(out_arrs[0]).reshape(B, M, C).astype(np.float32)
